# revision 1
# baseline (speedup 1.0000x reference)
"""GAT (2-layer, 8-head) Trainium2 Bass kernel, SPMD over 8 NeuronCores.

Strategy (dst-node-parallel with replicated projection):
  - Host: sort edges by dst, give each core the edges whose dst falls in its
    1/8 node range. Precompute fused projection matrices Wcat = [W_heads | a_si | a_sj]
    so one matmul produces per-node head embeddings AND both attention score halves.
  - Device, per core (single NEFF):
      Phase A: project ALL nodes (replicated): table1[n] = [h1 (H*D) | sj | si] bf16,
               plus a 256B-row si-table for dst-side gathers.
      Phase B: for each owned 128-node tile: dma_gather rows by src and by dst,
               softmax (no max-subtraction needed: logits are O(5)), aggregate via
               onehot-matmul into PSUM (denominator = extra matmul columns),
               head-mean + ELU -> emb1 tile.
      Phase C: AllGather emb1 (bf16, 2.5MB).
      Phase D/E: same for layer 2 -> per-core partial graph-sum [1, 128].
  - Host: sum partials, mean, LayerNorm + 3-layer MLP head -> [16].
"""

import numpy as np
import ml_dtypes

BF16 = ml_dtypes.bfloat16

# problem sizes (fixed for this problem)
N_NODES = 10000
N_EDGES = 160000
N_FEAT = 512
HEADS = 8
HID = 128
OUT = 16

N_CORES = 8
P = 128


# ----------------------------------------------------------------------------
# host-side prep
# ----------------------------------------------------------------------------

def _wrap_idx(idx_i16: np.ndarray) -> np.ndarray:
    """[n] int16 -> [128, n//16] wrapped layout for dma_gather (16-partition
    wrap, replicated 8x down the partitions)."""
    n = idx_i16.shape[0]
    assert n % 16 == 0
    w = idx_i16.reshape(n // 16, 16).T  # [16, n//16]
    return np.tile(w, (8, 1))  # [128, n//16]


def host_prep(node_features, edge_src, edge_dst, W1, a1, W2, a2):
    """Returns (in_maps, meta). meta carries static structure for the builder."""
    nloc = N_NODES // N_CORES  # 1250
    T = -(-nloc // P)  # owned tiles per core (10)

    order = np.argsort(edge_dst, kind="stable")
    src_s = edge_src[order].astype(np.int64)
    dst_s = edge_dst[order].astype(np.int64)

    # per (core, tile) edge ranges
    core_lo = np.searchsorted(dst_s, np.arange(N_CORES) * nloc, side="left")
    core_hi = np.searchsorted(dst_s, (np.arange(N_CORES) + 1) * nloc, side="left")

    # common chunk count per tile position (same across cores; program is SPMD)
    tile_cnt = np.zeros((N_CORES, T), dtype=np.int64)
    for c in range(N_CORES):
        base = c * nloc
        for t in range(T):
            n0 = base + t * P
            n1 = min(base + (t + 1) * P, (c + 1) * nloc)
            lo = np.searchsorted(dst_s, n0, side="left")
            hi = np.searchsorted(dst_s, n1, side="left")
            tile_cnt[c, t] = hi - lo
    C_t = [int(-(-tile_cnt[:, t].max() // P)) for t in range(T)]  # chunks per tile pos

    SLOTS = [c * P for c in C_t]
    tot_chunks = sum(C_t)
    tot_slots = tot_chunks * P

    in_maps = []
    xbf = np.ascontiguousarray(node_features.astype(BF16))

    # fused projection matrices
    def wcat(W, a, K):
        w = np.transpose(W, (2, 0, 1)).reshape(K, HEADS * HID)  # [K, H*D]
        si = np.einsum("hdf,hd->fh", W, a[:, :HID])  # [K, H]  (dst-side score)
        sj = np.einsum("hdf,hd->fh", W, a[:, HID:])  # [K, H]  (src-side score)
        return np.concatenate([w, sj, si], axis=1).astype(BF16)  # [K, H*D+16]

    w1c = np.ascontiguousarray(wcat(W1, a1, N_FEAT))
    w2c = np.ascontiguousarray(wcat(W2, a2, HID))

    for c in range(N_CORES):
        base = c * nloc
        src_pad = np.zeros(tot_slots, dtype=np.int64)
        dst_pad = np.zeros(tot_slots, dtype=np.int64)
        dstl_pad = np.full(tot_slots, 30000.0, dtype=np.float32)
        off = 0
        for t in range(T):
            n0 = base + t * P
            n1 = min(base + (t + 1) * P, (c + 1) * nloc)
            lo = np.searchsorted(dst_s, n0, side="left")
            hi = np.searchsorted(dst_s, n1, side="left")
            k = hi - lo
            sl = SLOTS[t]
            src_pad[off:off + k] = src_s[lo:hi]
            dst_pad[off:off + k] = dst_s[lo:hi]
            dstl_pad[off:off + k] = (dst_s[lo:hi] - n0).astype(np.float32)
            # padding: src/dst index 0 (valid gather), dstl 30000 (never matches)
            off += sl
        assert off == tot_slots

        # wrapped int16 gather indices, per tile concatenated along free dim
        src16 = np.concatenate(
            [_wrap_idx(src_pad[sum(SLOTS[:t]):sum(SLOTS[:t + 1])].astype(np.int16))
             for t in range(T)], axis=1)
        dst16 = np.concatenate(
            [_wrap_idx(dst_pad[sum(SLOTS[:t]):sum(SLOTS[:t + 1])].astype(np.int16))
             for t in range(T)], axis=1)
        # dstl columns: [128, tot_chunks]; element [p, g] = dstl of slot g*128+p
        dstl_cols = np.ascontiguousarray(
            dstl_pad.reshape(tot_chunks, P).T.astype(np.float32))

        in_maps.append({
            "xbf": xbf,
            "w1c": w1c,
            "w2c": w2c,
            "src16": np.ascontiguousarray(src16),
            "dst16": np.ascontiguousarray(dst16),
            "dstl": dstl_cols,
        })

    meta = {
        "T": T, "C_t": C_t, "nloc": nloc,
        "tot_chunks": tot_chunks,
    }
    return in_maps, meta


# ----------------------------------------------------------------------------
# device program
# ----------------------------------------------------------------------------

HD = HEADS * HID          # 1024
ROW = HD + P              # 1152 bf16 cols per table row  (2304 B, %256)
SI_ROW = P                # 128 bf16 cols per si-table row (256 B)
PCOLS = HD + 16           # projection output cols: [h (1024) | sj 8 | si 8]


def build_program(meta, debug=False, stages=5, iters=1, ablate=""):
    import concourse.bacc as bacc
    import concourse.mybir as mybir
    import concourse.tile as tile
    from concourse.library_config import mlp

    dt = mybir.dt
    Alu = mybir.AluOpType
    Act = mybir.ActivationFunctionType

    T = meta["T"]
    C_t = meta["C_t"]
    nloc = meta["nloc"]
    tot_chunks = meta["tot_chunks"]

    NT = -(-N_NODES // P)  # 79 node tiles overall
    K1 = N_FEAT // P       # 4 contraction chunks layer 1
    RB = 512               # rows per dma-transpose block

    nc = bacc.Bacc("TRN2", num_devices=N_CORES, num_swdge_queues=2,
                   dynamic_dma_scratch_size=49152)

    xbf = nc.dram_tensor("xbf", [N_NODES, N_FEAT], dt.bfloat16, kind="ExternalInput")
    w1c = nc.dram_tensor("w1c", [N_FEAT, PCOLS], dt.bfloat16, kind="ExternalInput")
    w2c = nc.dram_tensor("w2c", [HID, PCOLS], dt.bfloat16, kind="ExternalInput")
    src16 = nc.dram_tensor("src16", [P, tot_chunks * 8], dt.int16, kind="ExternalInput")
    dst16 = nc.dram_tensor("dst16", [P, tot_chunks * 8], dt.int16, kind="ExternalInput")
    dstl = nc.dram_tensor("dstl", [P, tot_chunks], dt.float32, kind="ExternalInput")

    out_vec = nc.dram_tensor("out_vec", [1, HID], dt.float32, kind="ExternalOutput")
    dbg = {}
    if debug:
        dbg["emb1"] = nc.dram_tensor("dbg_emb1", [N_NODES, HID], dt.float32,
                                     kind="ExternalOutput")
        dbg["tab1"] = nc.dram_tensor("dbg_tab1", [P, ROW], dt.bfloat16,
                                     kind="ExternalOutput")
        dbg["embloc"] = nc.dram_tensor("dbg_embloc", [nloc, HID], dt.bfloat16,
                                       kind="ExternalOutput")

    table1 = nc.dram_tensor("table1", [N_NODES, ROW], dt.bfloat16)
    sit1 = nc.dram_tensor("sit1", [N_NODES, SI_ROW], dt.bfloat16)
    table2 = nc.dram_tensor("table2", [N_NODES, ROW], dt.bfloat16)
    sit2 = nc.dram_tensor("sit2", [N_NODES, SI_ROW], dt.bfloat16)
    emb_loc = nc.dram_tensor("emb_loc", [nloc, HID], dt.bfloat16)
    emb_all = nc.dram_tensor("emb_all", [N_NODES, HID], dt.bfloat16,
                             addr_space="Shared")

    with tile.TileContext(nc) as tc:
        with (
            tc.tile_pool(name="const", bufs=1) as cpool,
            tc.tile_pool(name="xtp", bufs=8) as tpool,
            tc.tile_pool(name="work", bufs=2) as wpool,
            tc.tile_pool(name="chunk", bufs=4) as kpool,
            tc.tile_pool(name="psum", bufs=2, space="PSUM") as pspool,
            tc.tile_pool(name="psg", bufs=1, space="PSUM") as psg,
        ):
            nc.gpsimd.load_library(mlp)

            # ---- constants ----
            iota_i = cpool.tile([P, P], dt.int32)
            nc.gpsimd.iota(iota_i[:], pattern=[[1, P]], base=0, channel_multiplier=0)
            iota_bf = cpool.tile([P, P], dt.bfloat16)
            nc.vector.tensor_copy(iota_bf[:], iota_i[:])
            ones_col = cpool.tile([P, 1], dt.bfloat16)
            nc.gpsimd.memset(ones_col[:], 1.0)

            w1s = cpool.tile([P, K1 * PCOLS], dt.bfloat16)
            nc.sync.dma_start(
                out=w1s[:].rearrange("p (k c) -> p k c", k=K1),
                in_=w1c[:].rearrange("(k p) c -> p k c", p=P))
            w2s = cpool.tile([P, PCOLS], dt.bfloat16)
            nc.sync.dma_start(out=w2s[:], in_=w2c[:])

            srcI = cpool.tile([P, tot_chunks * 8], dt.int16)
            nc.sync.dma_start(out=srcI[:], in_=src16[:])
            dstI = cpool.tile([P, tot_chunks * 8], dt.int16)
            nc.sync.dma_start(out=dstI[:], in_=dst16[:])
            dstlS = cpool.tile([P, tot_chunks], dt.float32)
            nc.sync.dma_start(out=dstlS[:], in_=dstl[:])

            # ---------------- projection phase ----------------
            def project(src_dram, wtile, K, table, sit):
                """table[n] = [h | sj | si] for all nodes; sit[n] = [si]."""
                nblk = -(-N_NODES // RB)
                for b in range(nblk):
                    r0 = b * RB
                    rn = min(RB, N_NODES - r0)
                    xT = []
                    for k in range(K):
                        t_ = tpool.tile([P, RB], dt.bfloat16, tag="xT")
                        eng = nc.sync if k % 2 == 0 else nc.scalar
                        eng.dma_start_transpose(
                            t_[:, :rn], src_dram[r0:r0 + rn, k * P:(k + 1) * P])
                        xT.append(t_)
                    SB = RB // P
                    row = wpool.tile([P, SB, ROW], dt.bfloat16, tag="row")
                    sirow = wpool.tile([P, SB, SI_ROW], dt.bfloat16, tag="sirow")
                    for s in range(SB):
                        n0 = r0 + s * P
                        if n0 >= N_NODES:
                            break
                        nn = min(P, N_NODES - n0)
                        ps = pspool.tile([P, PCOLS], dt.float32, tag="ps")
                        for k in range(K):
                            lhsT = xT[k][:, s * P:s * P + nn]
                            rhs = wtile[:, k * PCOLS:(k + 1) * PCOLS] if K > 1 \
                                else wtile[:]
                            st, sp = (k == 0), (k == K - 1)
                            nc.tensor.matmul(ps[:nn, 0:512], lhsT=lhsT,
                                             rhs=rhs[:, 0:512], start=st, stop=sp)
                            nc.tensor.matmul(ps[:nn, 512:1024], lhsT=lhsT,
                                             rhs=rhs[:, 512:1024], start=st, stop=sp)
                            nc.tensor.matmul(ps[:nn, 1024:PCOLS], lhsT=lhsT,
                                             rhs=rhs[:, 1024:PCOLS], start=st, stop=sp)
                        nc.vector.tensor_copy(row[:nn, s, 0:HD], ps[:nn, 0:HD])
                        # sj at cols 1024:1032 of the row
                        nc.vector.tensor_copy(row[:nn, s, HD:HD + 8],
                                              ps[:nn, HD:HD + 8])
                        nc.scalar.activation(sirow[:nn, s, 0:8],
                                             ps[:nn, HD + 8:HD + 16], Act.Copy)
                    if rn == RB:
                        nc.scalar.dma_start(
                            out=table[r0:r0 + RB, :].rearrange(
                                "(a p) c -> p a c", p=P),
                            in_=row[:])
                        nc.sync.dma_start(
                            out=sit[r0:r0 + RB, :].rearrange(
                                "(a p) c -> p a c", p=P),
                            in_=sirow[:])
                    else:
                        for s in range(SB):
                            n0 = r0 + s * P
                            if n0 >= N_NODES:
                                break
                            nn = min(P, N_NODES - n0)
                            nc.scalar.dma_start(out=table[n0:n0 + nn, :],
                                                in_=row[:nn, s, :])
                            nc.sync.dma_start(out=sit[n0:n0 + nn, :],
                                              in_=sirow[:nn, s, :])

            # ---------------- edge phase ----------------
            def edges(table, sit, layer):
                gps = psg.tile([1, HID], dt.float32, tag="gsum", name="gps") if layer == 2 else None
                ebuf = cpool.tile([P, T * HID], dt.bfloat16, name="ebuf") if layer == 2 else None
                for t in range(T):
                    C = C_t[t]
                    ioff = sum(C_t[:t])
                    n0t = t * P
                    nn_t = min(P, nloc - n0t)
                    G = wpool.tile([P, C, ROW], dt.bfloat16, tag="G")
                    SIG = wpool.tile([P, C, SI_ROW], dt.bfloat16, tag="SIG")
                    Ch = C // 2
                    if ablate != "nogather":
                        _gathers = True
                    else:
                        _gathers = False
                    if _gathers:
                        nc.gpsimd.dma_gather(
                        G[:, 0:Ch, :], table[:], srcI[:, ioff * 8:(ioff + Ch) * 8],
                        Ch * P, Ch * P, ROW, single_packet=False, queue_num=0)
                        nc.gpsimd.dma_gather(
                            G[:, Ch:C, :], table[:],
                            srcI[:, (ioff + Ch) * 8:(ioff + C) * 8],
                            (C - Ch) * P, (C - Ch) * P, ROW, single_packet=False,
                            queue_num=1)
                        nc.gpsimd.dma_gather(
                            SIG[:, 0:Ch, :], sit[:],
                            dstI[:, ioff * 8:(ioff + Ch) * 8],
                            Ch * P, Ch * P, SI_ROW, single_packet=False, queue_num=1)
                        nc.gpsimd.dma_gather(
                            SIG[:, Ch:C, :], sit[:],
                            dstI[:, (ioff + Ch) * 8:(ioff + C) * 8],
                            (C - Ch) * P, (C - Ch) * P, SI_ROW, single_packet=False,
                            queue_num=0)
                    ps = pspool.tile([P, PCOLS], dt.float32, tag="ps")
                    # batched attention logits for the whole tile
                    LG = wpool.tile([P, C * 8], dt.float32, tag="LG")
                    nc.vector.tensor_tensor(
                        out=LG[:].rearrange("p (c e) -> p c e", c=C),
                        in0=SIG[:, :, 0:8], in1=G[:, :, HD:HD + 8], op=Alu.add)
                    LR = wpool.tile([P, C * 8], dt.float32, tag="LR")
                    nc.vector.tensor_scalar_mul(LR[:], LG[:], 0.01)
                    MX = wpool.tile([P, C * 8], dt.float32, tag="MX")
                    nc.vector.tensor_tensor(out=MX[:], in0=LG[:], in1=LR[:],
                                            op=Alu.max)
                    EX = wpool.tile([P, C * 8], dt.float32, tag="EX")
                    nc.scalar.activation(EX[:], MX[:], Act.Exp)
                    EXbf = wpool.tile([P, C, 8], dt.bfloat16, tag="EXbf")
                    nc.vector.tensor_copy(
                        EXbf[:], EX[:].rearrange("p (c e) -> p c e", c=C))
                    for c in range(C):
                        msg = kpool.tile([P, HD], dt.bfloat16, tag="msg")
                        if ablate not in ("noscal", "nochunk"):
                            for h in range(HEADS):
                                if h in (3, 7):
                                    nc.scalar.activation(
                                        msg[:, h * HID:(h + 1) * HID],
                                        G[:, c, h * HID:(h + 1) * HID],
                                        Act.Copy,
                                        scale=EX[:, c * 8 + h:c * 8 + h + 1])
                                else:
                                    nc.vector.tensor_scalar_mul(
                                        msg[:, h * HID:(h + 1) * HID],
                                        G[:, c, h * HID:(h + 1) * HID],
                                        EX[:, c * 8 + h:c * 8 + h + 1])
                        oh = kpool.tile([P, P], dt.bfloat16, tag="oh")
                        if ablate not in ("nooh", "nochunk"):
                            nc.vector.tensor_scalar(
                                out=oh[:], in0=iota_bf[:],
                                scalar1=dstlS[:, ioff + c:ioff + c + 1], scalar2=None,
                                op0=Alu.is_equal)
                        lhs = iota_bf if ablate in ("nooh", "nochunk") else oh
                        st, sp = (c == 0), (c == C - 1)
                        if ablate not in ("nomm", "nochunk"):
                            nc.tensor.matmul(ps[:, 0:512], lhsT=lhs[:],
                                             rhs=msg[:, 0:512], start=st, stop=sp)
                            nc.tensor.matmul(ps[:, 512:1024], lhsT=lhs[:],
                                             rhs=msg[:, 512:1024], start=st, stop=sp)
                        nc.tensor.matmul(ps[:, 1024:1032], lhsT=lhs[:],
                                         rhs=EXbf[:, c, :], start=st, stop=sp)
                    # ---- postprocess tile ----
                    den = kpool.tile([P, 8], dt.float32, tag="den")
                    nc.vector.tensor_scalar(out=den[:], in0=ps[:, 1024:1032],
                                            scalar1=float(HEADS), scalar2=1e-30,
                                            op0=Alu.mult, op1=Alu.max)
                    rec = kpool.tile([P, 8], dt.float32, tag="rec")
                    nc.vector.reciprocal(rec[:], den[:])
                    Sa = kpool.tile([P, HID], dt.float32, tag="Sa")
                    Sb = kpool.tile([P, HID], dt.float32, tag="Sb")
                    nc.vector.tensor_scalar_mul(Sa[:], ps[:, 0:HID], rec[:, 0:1])
                    for h in range(1, HEADS):
                        tmp = kpool.tile([P, HID], dt.float32, tag="tmp")
                        nc.vector.tensor_scalar_mul(
                            tmp[:], ps[:, h * HID:(h + 1) * HID], rec[:, h:h + 1])
                        a, b = (Sa, Sb) if h % 2 == 1 else (Sb, Sa)
                        nc.vector.tensor_tensor(out=b[:], in0=a[:], in1=tmp[:],
                                                op=Alu.add)
                    S = Sb if HEADS % 2 == 0 else Sa  # final sum location
                    # elu(S) = exp(min(S,0)) - 1 + max(S,0)
                    neg = kpool.tile([P, HID], dt.float32, tag="neg")
                    nc.vector.tensor_scalar_min(neg[:], S[:], 0.0)
                    en = kpool.tile([P, HID], dt.float32, tag="en")
                    nc.scalar.activation(en[:], neg[:], Act.Exp)
                    pos = kpool.tile([P, HID], dt.float32, tag="pos")
                    nc.vector.tensor_scalar_max(pos[:], S[:], 0.0)
                    eadd = kpool.tile([P, HID], dt.float32, tag="eadd")
                    nc.vector.tensor_tensor(out=eadd[:], in0=en[:], in1=pos[:],
                                            op=Alu.add)
                    if layer == 1:
                        ebf = kpool.tile([P, HID], dt.bfloat16, tag="ebf")
                        nc.vector.tensor_scalar_add(ebf[:], eadd[:], -1.0)
                        nc.sync.dma_start(out=emb_loc[n0t:n0t + nn_t, :],
                                          in_=ebf[:nn_t, :])
                    else:
                        nc.vector.tensor_scalar_add(
                            ebuf[:, t * HID:(t + 1) * HID], eadd[:], -1.0)
                if layer == 2:
                    for t in range(T):
                        nn_t = min(P, nloc - t * P)
                        nc.tensor.matmul(gps[0:1, :], lhsT=ones_col[:nn_t, :],
                                         rhs=ebuf[:nn_t, t * HID:(t + 1) * HID],
                                         start=(t == 0), stop=(t == T - 1))
                return gps

            # ---------------- main flow ----------------
            def zero_out_vec():
                z = kpool.tile([1, HID], dt.float32, tag="gout", name="z")
                nc.gpsimd.memset(z[:], 0.0)
                nc.sync.dma_start(out=out_vec[:], in_=z[:])

            def flow():
                project(xbf, w1s, K1, table1, sit1)
                if debug:
                    tb0 = wpool.tile([P, ROW], dt.bfloat16, tag="row", name="tb0")
                    nc.sync.dma_start(out=tb0[:], in_=table1[0:P, :])
                    nc.sync.dma_start(out=dbg["tab1"][:], in_=tb0[:])
                if stages >= 2:
                    edges(table1, sit1, layer=1)
                    if debug:
                        for b in range(-(-nloc // P)):
                            n0 = b * P
                            nn = min(P, nloc - n0)
                            te = wpool.tile([P, HID], dt.bfloat16, tag="dbgb",
                                            name="te")
                            nc.sync.dma_start(out=te[:nn, :],
                                              in_=emb_loc[n0:n0 + nn, :])
                            nc.sync.dma_start(out=dbg["embloc"][n0:n0 + nn, :],
                                              in_=te[:nn, :])
                if stages >= 3:
                    nc.gpsimd.collective_compute(
                        "AllGather", Alu.bypass,
                        ins=[emb_loc[:]], outs=[emb_all[:]],
                        replica_groups=[list(range(N_CORES))])
                    if debug:
                        # copy emb_all -> dbg_emb1 via sbuf (bf16 -> f32)
                        for b in range(-(-N_NODES // P)):
                            n0 = b * P
                            nn = min(P, N_NODES - n0)
                            tt = wpool.tile([P, HID], dt.float32, tag="dbgt")
                            tb = wpool.tile([P, HID], dt.bfloat16, tag="dbgb")
                            nc.sync.dma_start(out=tb[:nn, :],
                                              in_=emb_all[n0:n0 + nn, :])
                            nc.vector.tensor_copy(tt[:nn, :], tb[:nn, :])
                            nc.sync.dma_start(out=dbg["emb1"][n0:n0 + nn, :],
                                              in_=tt[:nn, :])
                if stages >= 4:
                    project(emb_all, w2s, 1, table2, sit2)
                if stages == 6:
                    edges(table2, sit2, layer=1)
                if stages == 5:
                    gps = edges(table2, sit2, layer=2)
                    gout = kpool.tile([1, HID], dt.float32, tag="gout")
                    nc.vector.tensor_copy(gout[:], gps[:])
                    nc.sync.dma_start(out=out_vec[:], in_=gout[:])
                else:
                    zero_out_vec()

            for _it in range(iters):
                flow()

    nc.compile()
    return nc


# ----------------------------------------------------------------------------
# top-level kernel
# ----------------------------------------------------------------------------

_CACHE = {}


def _run_device(in_maps, meta):
    from concourse.bass_utils import run_bass_kernel_spmd
    key = "prog"
    if key not in _CACHE:
        _CACHE[key] = build_program(meta)
    nc = _CACHE[key]
    res = run_bass_kernel_spmd(nc, in_maps, core_ids=list(range(N_CORES)))
    return res


def host_finish(partials, ln_g, ln_b, Wl1, bl1, Wl2, bl2, Wl3, bl3):
    g = partials.sum(axis=0) / np.float32(N_NODES)  # [HID]
    mu = g.mean()
    var = ((g - mu) ** 2).mean()
    gn = (g - mu) / np.sqrt(var + 1e-5) * ln_g + ln_b
    x = Wl1 @ gn + bl1
    x = np.maximum(x, 0.01 * x)
    x = Wl2 @ x + bl2
    x = np.maximum(x, 0.01 * x)
    x = Wl3 @ x + bl3
    return np.maximum(x, 0.0).astype(np.float32)


def kernel(node_features, edge_src, edge_dst, W1, a1, W2, a2,
           ln_g, ln_b, Wl1, bl1, Wl2, bl2, Wl3, bl3):
    node_features = np.asarray(node_features, dtype=np.float32)
    edge_src = np.asarray(edge_src, dtype=np.int32)
    edge_dst = np.asarray(edge_dst, dtype=np.int32)
    in_maps, meta = host_prep(node_features, edge_src, edge_dst,
                              np.asarray(W1, np.float32), np.asarray(a1, np.float32),
                              np.asarray(W2, np.float32), np.asarray(a2, np.float32))
    res = _run_device(in_maps, meta)
    partials = np.stack([res.results[c]["out_vec"][0] for c in range(N_CORES)])
    return host_finish(partials.astype(np.float64),
                       np.asarray(ln_g, np.float64), np.asarray(ln_b, np.float64),
                       np.asarray(Wl1, np.float64), np.asarray(bl1, np.float64),
                       np.asarray(Wl2, np.float64), np.asarray(bl2, np.float64),
                       np.asarray(Wl3, np.float64), np.asarray(bl3, np.float64))



# revision 27
# speedup vs baseline: 1.2528x; 1.2528x over previous
"""GAT (2-layer, 8-head) Trainium2 Bass kernel, SPMD over 8 NeuronCores. v2.

Strategy (dst-node-parallel, fp8 message tables, host-built one-hot mats):
  - Host: sort edges by dst; each core owns 1250 dst nodes (10 tiles of 128).
    Per tile, edges are padded to an even number of 128-slot chunks.
    Host precomputes:
      * xT      [512, 10000] bf16  pretransposed features (proj1 lhsT)
      * w1c/w2c [K, 1040]    bf16  fused [W_heads | a_sj | a_si]
      * src16   wrapped int16 gather indices (edge src, per core)
      * siI     wrapped int16 indices of the core's owned nodes (si rows)
      * oh8     [128, tot_chunks*128] fp8  one-hot scatter-matmul lhsT:
                oh[p, g*128+d] = (dstl[slot g*128+p] == d)
      * oh2b    [128, tot_chunks*128] bf16 per-chunk transpose of oh (maps
                dst-side scores si onto edge slots via small matmuls)
  - Device per core (single NEFF):
      proj: table[n] = [h fp8 1024B | sj bf16 16B | pad] (1280B rows),
            sit[n] = 256B row, si in first 16B. Replicated over all nodes.
      edges per owned tile: dma_gather 1280B rows by src; si gathered once
            per layer (owned rows, partition-aligned); logits -> exp;
            alpha-weighted scatter via fp8 DoubleRow matmuls (2 chunks per
            instruction, denominator rides the same weights);
            head-mean + ELU -> emb_loc.
      AllGather emb_locT (locally transposed) -> emb_allT [1024, 1250].
      layer 2 -> per-core partial graph-sum [1, 128].
  - Host: sum partials, LayerNorm + MLP head -> [16].
"""

import numpy as np
import ml_dtypes

BF16 = ml_dtypes.bfloat16
F8 = ml_dtypes.float8_e4m3

N_NODES = 10000
N_EDGES = 160000
N_FEAT = 512
HEADS = 8
HID = 128
OUT = 16

N_CORES = 8
P = 128

HD = HEADS * HID            # 1024
ROWB = 1280                 # table row bytes: 1024 fp8 h + 16 B bf16 sj + pad
SIROW = 256                 # si table row: 256 fp8 = 256 B (16 used: hi|lo)
PCOLS = HD + 16             # proj out cols [h 1024 | sj 8 | si 8]


# ----------------------------------------------------------------------------
# host-side prep
# ----------------------------------------------------------------------------

def _wrap_idx(idx_i16: np.ndarray) -> np.ndarray:
    """[n] int16 -> [128, n//16] wrapped layout for dma_gather."""
    n = idx_i16.shape[0]
    assert n % 16 == 0
    w = idx_i16.reshape(n // 16, 16).T
    return np.tile(w, (8, 1))


def host_prep(node_features, edge_src, edge_dst, W1, a1, W2, a2):
    nloc = N_NODES // N_CORES  # 1250
    T = -(-nloc // P)          # 10

    order = np.argsort(edge_dst, kind="stable")
    src_s = edge_src[order].astype(np.int64)
    dst_s = edge_dst[order].astype(np.int64)

    tile_cnt = np.zeros((N_CORES, T), dtype=np.int64)
    for c in range(N_CORES):
        base = c * nloc
        for t in range(T):
            n0 = base + t * P
            n1 = min(base + (t + 1) * P, (c + 1) * nloc)
            lo = np.searchsorted(dst_s, n0, side="left")
            hi = np.searchsorted(dst_s, n1, side="left")
            tile_cnt[c, t] = hi - lo
    C_t = []
    for t in range(T):
        c_ = int(-(-tile_cnt[:, t].max() // P))
        C_t.append(c_ + (c_ % 2))  # even, for DoubleRow chunk pairs
    assert max(C_t) * 8 <= 176, f"C_t too large: {C_t}"

    SLOTS = [c * P for c in C_t]
    tot_chunks = sum(C_t)
    tot_slots = tot_chunks * P

    xT = np.ascontiguousarray(node_features.astype(np.float32).T.astype(BF16))

    def wcat(W, a, K):
        w = np.transpose(W, (2, 0, 1)).reshape(K, HEADS * HID)
        si = np.einsum("hdf,hd->fh", W, a[:, :HID])   # dst-side score
        sj = np.einsum("hdf,hd->fh", W, a[:, HID:])   # src-side score
        return np.ascontiguousarray(
            np.concatenate([w, sj, si], axis=1).astype(BF16))

    w1c = wcat(W1, a1, N_FEAT)
    w2c = wcat(W2, a2, HID)

    in_maps = []
    for c in range(N_CORES):
        base = c * nloc
        src_pad = np.zeros(tot_slots, dtype=np.int64)
        dstl_pad = np.full(tot_slots, 30000, dtype=np.int64)
        off = 0
        for t in range(T):
            n0 = base + t * P
            n1 = min(base + (t + 1) * P, (c + 1) * nloc)
            lo = np.searchsorted(dst_s, n0, side="left")
            hi = np.searchsorted(dst_s, n1, side="left")
            k = hi - lo
            sub = np.argsort(src_s[lo:hi], kind="stable")  # gather locality
            src_pad[off:off + k] = src_s[lo:hi][sub]
            dstl_pad[off:off + k] = (dst_s[lo:hi][sub] - n0)
            off += SLOTS[t]
        assert off == tot_slots

        src16 = np.concatenate(
            [_wrap_idx(src_pad[sum(SLOTS[:t]):sum(SLOTS[:t + 1])].astype(np.int16))
             for t in range(T)], axis=1)

        # owned-node si indices: slot t*128+d -> node base+t*128+d (pad -> 0;
        # pads must be VALID indices so gathered rows stay finite)
        own = np.where(np.arange(T * P) < nloc, base + np.arange(T * P), 0)
        siI = _wrap_idx(own.astype(np.int16))

        # one-hot matrices: slot s = chunk g * 128 + partition p
        dstl_pg = dstl_pad.reshape(tot_chunks, P)            # [g, p]
        dgrid = np.arange(P)
        oh = (dstl_pg[:, :, None] == dgrid[None, None, :])   # [g, p, d]
        oh8 = np.ascontiguousarray(
            oh.transpose(1, 0, 2).reshape(P, tot_chunks * P).astype(F8))
        oh28 = np.ascontiguousarray(
            oh.transpose(2, 0, 1).reshape(P, tot_chunks * P).astype(F8))

        in_maps.append({
            "xT": xT,
            "w1c": w1c,
            "w2c": w2c,
            "src16": np.ascontiguousarray(src16),
            "siI": np.ascontiguousarray(siI),
            "oh8": oh8,
            "oh28": oh28,
        })

    meta = {"T": T, "C_t": C_t, "nloc": nloc, "tot_chunks": tot_chunks}
    return in_maps, meta


# ----------------------------------------------------------------------------
# device program
# ----------------------------------------------------------------------------

def build_program(meta, debug=False, stages=5, iters=1, ablate=""):
    import concourse.bacc as bacc
    import concourse.mybir as mybir
    import concourse.tile as tile
    from concourse.library_config import mlp

    dt = mybir.dt
    Alu = mybir.AluOpType
    Act = mybir.ActivationFunctionType
    DR = mybir.MatmulPerfMode.DoubleRow

    T = meta["T"]
    C_t = meta["C_t"]
    nloc = meta["nloc"]
    tot_chunks = meta["tot_chunks"]

    K1 = N_FEAT // P  # 4
    RB = 512

    nc = bacc.Bacc("TRN2", num_devices=N_CORES, num_swdge_queues=2,
                   dynamic_dma_scratch_size=49152)

    xT = nc.dram_tensor("xT", [N_FEAT, N_NODES], dt.bfloat16,
                        kind="ExternalInput")
    w1c = nc.dram_tensor("w1c", [N_FEAT, PCOLS], dt.bfloat16,
                         kind="ExternalInput")
    w2c = nc.dram_tensor("w2c", [HID, PCOLS], dt.bfloat16,
                         kind="ExternalInput")
    src16 = nc.dram_tensor("src16", [P, tot_chunks * 8], dt.int16,
                           kind="ExternalInput")
    siI16 = nc.dram_tensor("siI", [P, T * 8], dt.int16, kind="ExternalInput")
    oh8 = nc.dram_tensor("oh8", [P, tot_chunks * P], dt.float8e4,
                         kind="ExternalInput")
    oh28 = nc.dram_tensor("oh28", [P, tot_chunks * P], dt.float8e4,
                          kind="ExternalInput")

    out_vec = nc.dram_tensor("out_vec", [1, HID], dt.float32,
                             kind="ExternalOutput")
    dbg = {}
    if debug:
        dbg["embloc"] = nc.dram_tensor("dbg_embloc", [nloc, HID], dt.float32,
                                       kind="ExternalOutput")
        dbg["tab"] = nc.dram_tensor("dbg_tab", [P, ROWB], dt.float32,
                                    kind="ExternalOutput")
        dbg["g"] = nc.dram_tensor("dbg_g", [P, HD], dt.float32,
                                  kind="ExternalOutput")
        dbg["sj"] = nc.dram_tensor("dbg_sj", [P, 8], dt.float32,
                                   kind="ExternalOutput")
        dbg["lg"] = nc.dram_tensor("dbg_lg", [P, 144], dt.float32,
                                   kind="ExternalOutput")
        dbg["ex"] = nc.dram_tensor("dbg_ex", [P, 144], dt.float32,
                                   kind="ExternalOutput")
        dbg["S"] = nc.dram_tensor("dbg_S", [P, HID], dt.float32,
                                  kind="ExternalOutput")
        dbg["sig"] = nc.dram_tensor("dbg_sig", [P, 16], dt.float32,
                                    kind="ExternalOutput")

    table1 = nc.dram_tensor("table1", [N_NODES, ROWB], dt.float8e4)
    sit1 = nc.dram_tensor("sit1", [N_NODES, SIROW], dt.float8e4)
    table2 = nc.dram_tensor("table2", [N_NODES, ROWB], dt.float8e4)
    sit2 = nc.dram_tensor("sit2", [N_NODES, SIROW], dt.float8e4)
    nloc_pad = T * P  # 1280: transpose blocks must be %16 rows
    emb_loc = nc.dram_tensor("emb_loc", [nloc_pad, HID], dt.bfloat16)
    emb_locT = nc.dram_tensor("emb_locT", [P, nloc], dt.bfloat16)
    emb_allT = nc.dram_tensor("emb_allT", [N_CORES * P, nloc], dt.bfloat16,
                              addr_space="Shared")

    with tile.TileContext(nc) as tc:
        with (
            tc.tile_pool(name="const", bufs=1) as cpool,
            tc.tile_pool(name="xtp", bufs=8) as tpool,
            tc.tile_pool(name="work", bufs=2) as wpool,
            tc.tile_pool(name="chunk", bufs=4) as kpool,
            tc.tile_pool(name="post", bufs=2) as opool,
            tc.tile_pool(name="psum", bufs=2, space="PSUM") as pspool,
            tc.tile_pool(name="psg", bufs=1, space="PSUM") as psg,
        ):
            nc.gpsimd.load_library(mlp)

            # ---- constants (loaded once per NEFF) ----
            ones_col = cpool.tile([P, 1], dt.bfloat16)
            nc.gpsimd.memset(ones_col[:], 1.0)
            nlog16 = cpool.tile([P, 1], dt.float32)
            nc.gpsimd.memset(nlog16[:], -2.772588722239781)

            w1s = cpool.tile([P, K1 * PCOLS], dt.bfloat16)
            nc.sync.dma_start(
                out=w1s[:].rearrange("p (k c) -> p k c", k=K1),
                in_=w1c[:].rearrange("(k p) c -> p k c", p=P))
            w2s = cpool.tile([P, PCOLS], dt.bfloat16)
            nc.sync.dma_start(out=w2s[:], in_=w2c[:])

            srcI = cpool.tile([P, tot_chunks * 8], dt.int16)
            nc.sync.dma_start(out=srcI[:], in_=src16[:])
            siIS = cpool.tile([P, T * 8], dt.int16)
            nc.sync.dma_start(out=siIS[:], in_=siI16[:])
            ohS = cpool.tile([P, tot_chunks * P], dt.float8e4)
            nc.scalar.dma_start(out=ohS[:], in_=oh8[:])
            oh2S = cpool.tile([P, tot_chunks * P], dt.float8e4)
            nc.scalar.dma_start(out=oh2S[:], in_=oh28[:])

            # ---------------- projection ----------------
            def load_lhsT_l2(dst, r0, rn):
                a = r0
                while a < r0 + rn:
                    cblk = a // nloc
                    e = min(r0 + rn, (cblk + 1) * nloc)
                    nc.sync.dma_start(
                        out=dst[:, a - r0:e - r0],
                        in_=emb_allT[cblk * P:(cblk + 1) * P,
                                     a - cblk * nloc:e - cblk * nloc])
                    a = e

            def project(layer, table, sit):
                K = K1 if layer == 1 else 1
                wtile = w1s if layer == 1 else w2s
                nblk = -(-N_NODES // RB)
                for b in range(nblk):
                    r0 = b * RB
                    rn = min(RB, N_NODES - r0)
                    xTs = []
                    for k in range(K):
                        t_ = tpool.tile([P, RB], dt.bfloat16, tag="xT")
                        if layer == 1:
                            eng = nc.sync if k % 2 == 0 else nc.scalar
                            eng.dma_start(out=t_[:, :rn],
                                          in_=xT[k * P:(k + 1) * P, r0:r0 + rn])
                        else:
                            load_lhsT_l2(t_, r0, rn)
                        xTs.append(t_)
                    SB = RB // P
                    row = wpool.tile([P, SB, ROWB], dt.float8e4, tag="row")
                    sirow = wpool.tile([P, SB, 16], dt.float8e4, tag="sirow")
                    for s in range(SB):
                        n0 = r0 + s * P
                        if n0 >= N_NODES:
                            break
                        nn = min(P, N_NODES - n0)
                        ps = pspool.tile([P, HD], dt.float32, tag="ps")
                        sd = pspool.tile([P, 192], dt.float32, tag="sd")
                        for k in range(K):
                            lhsT = xTs[k][:, s * P:s * P + nn]
                            rhs = wtile[:, k * PCOLS:(k + 1) * PCOLS] if K > 1 \
                                else wtile[:]
                            st, sp = (k == 0), (k == K - 1)
                            nc.tensor.matmul(ps[:nn, 0:512], lhsT=lhsT,
                                             rhs=rhs[:, 0:512], start=st, stop=sp)
                            nc.tensor.matmul(ps[:nn, 512:1024], lhsT=lhsT,
                                             rhs=rhs[:, 512:1024], start=st,
                                             stop=sp)
                            nc.tensor.matmul(sd[:nn, 0:16], lhsT=lhsT,
                                             rhs=rhs[:, 1024:1040], start=st,
                                             stop=sp)
                        nc.vector.tensor_copy(row[:nn, s, 0:768], ps[:nn, 0:768])
                        nc.scalar.activation(row[:nn, s, 768:1024],
                                             ps[:nn, 768:1024], Act.Copy)
                        nc.vector.tensor_copy(
                            row[:nn, s, 1024:1040].bitcast(dt.bfloat16),
                            sd[:nn, 0:8])
                        # si stored as fp8 hi + fp8 lo residual (full
                        # precision across two accumulating matmuls)
                        nc.vector.tensor_copy(sirow[:nn, s, 0:8], sd[:nn, 8:16])
                        hi32 = kpool.tile([P, 8], dt.float32, tag="hi32")
                        nc.vector.tensor_copy(hi32[:nn, :], sirow[:nn, s, 0:8])
                        lo32 = kpool.tile([P, 8], dt.float32, tag="lo32")
                        nc.vector.tensor_tensor(out=lo32[:nn, :],
                                                in0=sd[:nn, 8:16],
                                                in1=hi32[:nn, :],
                                                op=Alu.subtract)
                        nc.vector.tensor_copy(sirow[:nn, s, 8:16], lo32[:nn, :])
                    if rn == RB:
                        nc.scalar.dma_start(
                            out=table[r0:r0 + RB, :].rearrange(
                                "(a p) c -> p a c", p=P),
                            in_=row[:])
                        nc.sync.dma_start(
                            out=sit[r0:r0 + RB, 0:16].rearrange(
                                "(a p) c -> p a c", p=P),
                            in_=sirow[:, :, 0:16])
                    else:
                        for s in range(SB):
                            n0 = r0 + s * P
                            if n0 >= N_NODES:
                                break
                            nn = min(P, N_NODES - n0)
                            nc.scalar.dma_start(out=table[n0:n0 + nn, :],
                                                in_=row[:nn, s, :])
                            nc.sync.dma_start(out=sit[n0:n0 + nn, 0:16],
                                              in_=sirow[:nn, s, 0:16])

            # ---------------- edge phase ----------------
            def edges(table, sit, layer):
                gps = psg.tile([1, HID], dt.float32, tag="gsum", name="gps") \
                    if layer == 2 else None
                ebuf = cpool.tile([P, T * HID], dt.bfloat16, name="ebuf") \
                    if layer == 2 else None

                # owned-node si rows, partition-aligned: siG[d, t, :] is the
                # si row of node base + t*128 + d (fp8: cols 0:8 hi, 8:16 lo)
                siG = wpool.tile([P, T, SIROW], dt.float8e4, tag="siG")
                nc.gpsimd.dma_gather(
                    siG[:], sit[:], siIS[:], T * P, T * P, SIROW,
                    single_packet=False, queue_num=0)
                if debug and layer == 1:
                    dsg = opool.tile([P, 16], dt.float32, tag="dsg", name="dsg")
                    nc.vector.tensor_copy(dsg[:], siG[:, 0, 0:16])
                    nc.sync.dma_start(out=dbg["sig"][:], in_=dsg[:])

                for t in range(T):
                    C = C_t[t]
                    ioff = sum(C_t[:t])
                    n0t = t * P
                    nn_t = min(P, nloc - n0t)
                    G = wpool.tile([P, C, ROWB], dt.float8e4, tag="G")
                    if ablate != "nogather":
                        Ch = C // 2
                        nc.gpsimd.dma_gather(
                            G[:, 0:Ch, :], table[:],
                            srcI[:, ioff * 8:(ioff + Ch) * 8],
                            Ch * P, Ch * P, ROWB, single_packet=False,
                            queue_num=0)
                        nc.gpsimd.dma_gather(
                            G[:, Ch:C, :], table[:],
                            srcI[:, (ioff + Ch) * 8:(ioff + C) * 8],
                            (C - Ch) * P, (C - Ch) * P, ROWB,
                            single_packet=False, queue_num=1)

                    sd = pspool.tile([P, 192], dt.float32, tag="sd")
                    # distribute dst-side si onto edge slots: per chunk,
                    # sd[slot, c*8:(c+1)*8] = oh2_c.T @ (si_hi + si_lo)
                    for c in range(C):
                        nc.tensor.matmul(
                            sd[:, c * 8:(c + 1) * 8],
                            lhsT=oh2S[:, (ioff + c) * P:(ioff + c + 1) * P],
                            rhs=siG[:, t, 0:8], start=True, stop=False)
                        nc.tensor.matmul(
                            sd[:, c * 8:(c + 1) * 8],
                            lhsT=oh2S[:, (ioff + c) * P:(ioff + c + 1) * P],
                            rhs=siG[:, t, 8:16], start=False, stop=True)
                    # logits = si + sj; exp(leaky_relu(.)) — logits are O(5),
                    # no max-subtraction needed
                    LG = wpool.tile([P, C, 8], dt.float32, tag="LG")
                    nc.vector.tensor_tensor(
                        out=LG[:],
                        in0=sd[:, 0:C * 8].rearrange("p (c e) -> p c e", c=C),
                        in1=G[:, :, 1024:1040].bitcast(dt.bfloat16),
                        op=Alu.add)
                    MX = wpool.tile([P, C * 8], dt.float32, tag="MX")
                    nc.scalar.activation(MX[:],
                                         LG[:].rearrange("p c e -> p (c e)"),
                                         Act.Lrelu, alpha=0.01)
                    # exp scaled by 1/16 so msg = EX*G stays under the fp8
                    # e4m3 max (240); alpha normalization cancels the scale
                    EX = wpool.tile([P, C * 8], dt.float32, tag="EX")
                    nc.scalar.activation(EX[:], MX[:], Act.Exp,
                                         bias=nlog16[:])
                    EXf8 = wpool.tile([P, C, 8], dt.float8e4, tag="EXf8")
                    nc.vector.tensor_copy(
                        EXf8[:], EX[:].rearrange("p (c e) -> p c e", c=C))
                    if debug and layer == 1 and t == 0:
                        dg = wpool.tile([P, HD], dt.float32, tag="dg", name="dg")
                        nc.vector.tensor_copy(dg[:], G[:, 0, 0:HD])
                        nc.sync.dma_start(out=dbg["g"][:], in_=dg[:])
                        dsj = opool.tile([P, 8], dt.float32, tag="dsj",
                                         name="dsj")
                        nc.vector.tensor_copy(
                            dsj[:], G[:, 0, 1024:1040].bitcast(dt.bfloat16))
                        nc.sync.dma_start(out=dbg["sj"][:], in_=dsj[:])
                        dlg = wpool.tile([P, 144], dt.float32, tag="dlg",
                                         name="dlg")
                        nc.vector.tensor_copy(
                            dlg[:], LG[:].rearrange("p c e -> p (c e)")[:, 0:144])
                        nc.sync.dma_start(out=dbg["lg"][:], in_=dlg[:])
                        dex = wpool.tile([P, 144], dt.float32, tag="dex",
                                         name="dex")
                        nc.vector.tensor_copy(dex[:], EX[:, 0:144])
                        nc.sync.dma_start(out=dbg["ex"][:], in_=dex[:])

                    ps = pspool.tile([P, HD], dt.float32, tag="ps")
                    for cp in range(C // 2):
                        c0 = 2 * cp
                        msg = kpool.tile([P, 2, HD], dt.float8e4, tag="msg")
                        if ablate != "nomsg":
                            for j in (0, 1):
                                c = c0 + j
                                for h in range(HEADS):
                                    if h in (3, 7):
                                        nc.scalar.activation(
                                            msg[:, j, h * HID:(h + 1) * HID],
                                            G[:, c, h * HID:(h + 1) * HID],
                                            Act.Copy,
                                            scale=EX[:, c * 8 + h:c * 8 + h + 1])
                                    else:
                                        nc.vector.tensor_scalar_mul(
                                            msg[:, j, h * HID:(h + 1) * HID],
                                            G[:, c, h * HID:(h + 1) * HID],
                                            EX[:, c * 8 + h:c * 8 + h + 1])
                        ohp = ohS[:, (ioff + c0) * P:(ioff + c0 + 2) * P] \
                            .rearrange("p (two d) -> p two d", two=2)
                        st, sp = (cp == 0), (cp == C // 2 - 1)
                        if ablate != "nomm":
                            nc.tensor.matmul(ps[:, 0:512], lhsT=ohp,
                                             rhs=msg[:, :, 0:512], start=st,
                                             stop=sp, perf_mode=DR)
                            nc.tensor.matmul(ps[:, 512:1024], lhsT=ohp,
                                             rhs=msg[:, :, 512:1024], start=st,
                                             stop=sp, perf_mode=DR)
                        nc.tensor.matmul(sd[:, 176:184], lhsT=ohp,
                                         rhs=EXf8[:, c0:c0 + 2, :], start=st,
                                         stop=sp, perf_mode=DR)

                    # ---- postprocess tile ----
                    den = opool.tile([P, 8], dt.float32, tag="den")
                    nc.vector.tensor_scalar(out=den[:], in0=sd[:, 176:184],
                                            scalar1=float(HEADS), scalar2=1e-30,
                                            op0=Alu.mult, op1=Alu.max)
                    rec = opool.tile([P, 8], dt.float32, tag="rec")
                    nc.vector.reciprocal(rec[:], den[:])
                    Sa = opool.tile([P, HID], dt.float32, tag="Sa")
                    Sb = opool.tile([P, HID], dt.float32, tag="Sb")
                    nc.vector.tensor_scalar_mul(Sa[:], ps[:, 0:HID], rec[:, 0:1])
                    for h in range(1, HEADS):
                        tmp = opool.tile([P, HID], dt.float32, tag="tmp")
                        nc.vector.tensor_scalar_mul(
                            tmp[:], ps[:, h * HID:(h + 1) * HID], rec[:, h:h + 1])
                        a, b = (Sa, Sb) if h % 2 == 1 else (Sb, Sa)
                        nc.vector.tensor_tensor(out=b[:], in0=a[:], in1=tmp[:],
                                                op=Alu.add)
                    S = Sb if HEADS % 2 == 0 else Sa
                    if debug and layer == 1 and t == 0:
                        dS = wpool.tile([P, HID], dt.float32, tag="dS",
                                        name="dS")
                        nc.vector.tensor_copy(dS[:], S[:])
                        nc.sync.dma_start(out=dbg["S"][:], in_=dS[:])
                    # elu(S) = exp(min(S,0)) - 1 + max(S,0)
                    neg = opool.tile([P, HID], dt.float32, tag="neg")
                    nc.vector.tensor_scalar_min(neg[:], S[:], 0.0)
                    en = opool.tile([P, HID], dt.float32, tag="en")
                    nc.scalar.activation(en[:], neg[:], Act.Exp)
                    pos = opool.tile([P, HID], dt.float32, tag="pos")
                    nc.vector.tensor_scalar_max(pos[:], S[:], 0.0)
                    eadd = opool.tile([P, HID], dt.float32, tag="eadd")
                    nc.vector.tensor_tensor(out=eadd[:], in0=en[:], in1=pos[:],
                                            op=Alu.add)
                    if layer == 1:
                        ebf = opool.tile([P, HID], dt.bfloat16, tag="ebf")
                        nc.vector.tensor_scalar_add(ebf[:], eadd[:], -1.0)
                        nc.sync.dma_start(out=emb_loc[n0t:n0t + nn_t, :],
                                          in_=ebf[:nn_t, :])
                    else:
                        nc.vector.tensor_scalar_add(
                            ebuf[:, t * HID:(t + 1) * HID], eadd[:], -1.0)
                if layer == 2:
                    for t in range(T):
                        nn_t = min(P, nloc - t * P)
                        nc.tensor.matmul(gps[0:1, :], lhsT=ones_col[:nn_t, :],
                                         rhs=ebuf[:nn_t, t * HID:(t + 1) * HID],
                                         start=(t == 0), stop=(t == T - 1))
                return gps

            # ---------------- main flow ----------------
            def zero_out_vec():
                z = opool.tile([1, HID], dt.float32, tag="gout", name="z")
                nc.gpsimd.memset(z[:], 0.0)
                nc.sync.dma_start(out=out_vec[:], in_=z[:])

            def flow():
                project(1, table1, sit1)
                if debug:
                    tf8 = wpool.tile([P, ROWB], dt.float8e4, tag="tf8",
                                     name="tf8")
                    nc.sync.dma_start(out=tf8[:], in_=table1[0:P, :])
                    t32 = wpool.tile([P, ROWB], dt.float32, tag="t32",
                                     name="t32")
                    nc.vector.tensor_copy(t32[:], tf8[:])
                    nc.sync.dma_start(out=dbg["tab"][:], in_=t32[:])
                if stages >= 2:
                    edges(table1, sit1, layer=1)
                    if debug:
                        for b in range(-(-nloc // P)):
                            n0 = b * P
                            nn = min(P, nloc - n0)
                            te = wpool.tile([P, HID], dt.float32, tag="dbgt")
                            tb = wpool.tile([P, HID], dt.bfloat16, tag="dbgb")
                            nc.sync.dma_start(out=tb[:nn, :],
                                              in_=emb_loc[n0:n0 + nn, :])
                            nc.vector.tensor_copy(te[:nn, :], tb[:nn, :])
                            nc.sync.dma_start(out=dbg["embloc"][n0:n0 + nn, :],
                                              in_=te[:nn, :])
                if stages >= 3:
                    # local transpose emb_loc -> emb_locT, then AllGather
                    trh = wpool.tile([P, nloc_pad], dt.bfloat16, tag="trh")
                    for r0 in range(0, nloc_pad, RB):
                        rn = min(RB, nloc_pad - r0)
                        nc.sync.dma_start_transpose(trh[:, r0:r0 + rn],
                                                    emb_loc[r0:r0 + rn, :])
                    nc.scalar.dma_start(out=emb_locT[:], in_=trh[:, 0:nloc])
                    nc.gpsimd.collective_compute(
                        "AllGather", Alu.bypass,
                        ins=[emb_locT[:]], outs=[emb_allT[:]],
                        replica_groups=[list(range(N_CORES))])
                if stages >= 4:
                    project(2, table2, sit2)
                if stages >= 5:
                    gps = edges(table2, sit2, layer=2)
                    gout = opool.tile([1, HID], dt.float32, tag="gout")
                    nc.vector.tensor_copy(gout[:], gps[:])
                    nc.sync.dma_start(out=out_vec[:], in_=gout[:])
                else:
                    zero_out_vec()

            for _it in range(iters):
                flow()

    nc.compile()
    return nc


# ----------------------------------------------------------------------------
# top-level kernel
# ----------------------------------------------------------------------------

_CACHE = {}


def _run_device(in_maps, meta):
    from concourse.bass_utils import run_bass_kernel_spmd
    key = "prog"
    if key not in _CACHE:
        _CACHE[key] = build_program(meta)
    nc = _CACHE[key]
    res = run_bass_kernel_spmd(nc, in_maps, core_ids=list(range(N_CORES)))
    return res


def host_finish(partials, ln_g, ln_b, Wl1, bl1, Wl2, bl2, Wl3, bl3):
    g = partials.sum(axis=0) / np.float64(N_NODES)
    mu = g.mean()
    var = ((g - mu) ** 2).mean()
    gn = (g - mu) / np.sqrt(var + 1e-5) * ln_g + ln_b
    x = Wl1 @ gn + bl1
    x = np.maximum(x, 0.01 * x)
    x = Wl2 @ x + bl2
    x = np.maximum(x, 0.01 * x)
    x = Wl3 @ x + bl3
    return np.maximum(x, 0.0).astype(np.float32)


def kernel(node_features, edge_src, edge_dst, W1, a1, W2, a2,
           ln_g, ln_b, Wl1, bl1, Wl2, bl2, Wl3, bl3):
    node_features = np.asarray(node_features, dtype=np.float32)
    edge_src = np.asarray(edge_src, dtype=np.int32)
    edge_dst = np.asarray(edge_dst, dtype=np.int32)
    in_maps, meta = host_prep(node_features, edge_src, edge_dst,
                              np.asarray(W1, np.float32), np.asarray(a1, np.float32),
                              np.asarray(W2, np.float32), np.asarray(a2, np.float32))
    res = _run_device(in_maps, meta)
    partials = np.stack([res.results[c]["out_vec"][0] for c in range(N_CORES)])
    return host_finish(partials.astype(np.float64),
                       np.asarray(ln_g, np.float64), np.asarray(ln_b, np.float64),
                       np.asarray(Wl1, np.float64), np.asarray(bl1, np.float64),
                       np.asarray(Wl2, np.float64), np.asarray(bl2, np.float64),
                       np.asarray(Wl3, np.float64), np.asarray(bl3, np.float64))


# revision 35
# speedup vs baseline: 1.3985x; 1.1163x over previous
"""GAT (2-layer, 8-head) Trainium2 Bass kernel, SPMD over 8 NeuronCores. v2.

Strategy (dst-node-parallel, fp8 message tables, host-built one-hot mats):
  - Host: sort edges by dst; each core owns 1250 dst nodes (10 tiles of 128).
    Per tile, edges are padded to an even number of 128-slot chunks.
    Host precomputes:
      * xT      [512, 10000] bf16  pretransposed features (proj1 lhsT)
      * w1c/w2c [K, 1040]    bf16  fused [W_heads | a_sj | a_si]
      * src16   wrapped int16 gather indices (edge src, per core)
      * siI     wrapped int16 indices of the core's owned nodes (si rows)
      * oh8     [128, tot_chunks*128] fp8  one-hot scatter-matmul lhsT:
                oh[p, g*128+d] = (dstl[slot g*128+p] == d)
      * oh2b    [128, tot_chunks*128] bf16 per-chunk transpose of oh (maps
                dst-side scores si onto edge slots via small matmuls)
  - Device per core (single NEFF):
      proj: table[n] = [h fp8 1024B | sj bf16 16B | pad] (1280B rows),
            sit[n] = 256B row, si in first 16B. Replicated over all nodes.
      edges per owned tile: dma_gather 1280B rows by src; si gathered once
            per layer (owned rows, partition-aligned); logits -> exp;
            alpha-weighted scatter via fp8 DoubleRow matmuls (2 chunks per
            instruction, denominator rides the same weights);
            head-mean + ELU -> emb_loc.
      AllGather emb_locT (locally transposed) -> emb_allT [1024, 1250].
      layer 2 -> per-core partial graph-sum [1, 128].
  - Host: sum partials, LayerNorm + MLP head -> [16].
"""

import numpy as np
import ml_dtypes

BF16 = ml_dtypes.bfloat16
F8 = ml_dtypes.float8_e4m3

N_NODES = 10000
N_EDGES = 160000
N_FEAT = 512
HEADS = 8
HID = 128
OUT = 16

N_CORES = 8
P = 128

HD = HEADS * HID            # 1024
ROWB = 1280                 # table row bytes: 1024 fp8 h + 16 B bf16 sj + pad
SIROW = 256                 # si table row: 256 fp8 = 256 B (16 used: hi|lo)
PCOLS = HD + 16             # proj out cols [h 1024 | sj 8 | si 8]


# ----------------------------------------------------------------------------
# host-side prep
# ----------------------------------------------------------------------------

def _wrap_idx(idx_i16: np.ndarray) -> np.ndarray:
    """[n] int16 -> [128, n//16] wrapped layout for dma_gather."""
    n = idx_i16.shape[0]
    assert n % 16 == 0
    w = idx_i16.reshape(n // 16, 16).T
    return np.tile(w, (8, 1))


def host_prep(node_features, edge_src, edge_dst, W1, a1, W2, a2):
    nloc = N_NODES // N_CORES  # 1250
    T = -(-nloc // P)          # 10

    order = np.argsort(edge_dst, kind="stable")
    src_s = edge_src[order].astype(np.int64)
    dst_s = edge_dst[order].astype(np.int64)

    tile_cnt = np.zeros((N_CORES, T), dtype=np.int64)
    for c in range(N_CORES):
        base = c * nloc
        for t in range(T):
            n0 = base + t * P
            n1 = min(base + (t + 1) * P, (c + 1) * nloc)
            lo = np.searchsorted(dst_s, n0, side="left")
            hi = np.searchsorted(dst_s, n1, side="left")
            tile_cnt[c, t] = hi - lo
    C_t = []
    for t in range(T):
        c_ = int(-(-tile_cnt[:, t].max() // P))
        C_t.append(c_ + (c_ % 2))  # even, for DoubleRow chunk pairs
    assert max(C_t) * 8 <= 176, f"C_t too large: {C_t}"

    SLOTS = [c * P for c in C_t]
    tot_chunks = sum(C_t)
    tot_slots = tot_chunks * P

    xT = np.ascontiguousarray(node_features.astype(np.float32).T.astype(BF16))

    def wcat(W, a, K):
        w = np.transpose(W, (2, 0, 1)).reshape(K, HEADS * HID)
        si = np.einsum("hdf,hd->fh", W, a[:, :HID])   # dst-side score
        sj = np.einsum("hdf,hd->fh", W, a[:, HID:])   # src-side score
        return np.ascontiguousarray(
            np.concatenate([w, sj, si], axis=1).astype(BF16))

    w1c = wcat(W1, a1, N_FEAT)
    w2c = wcat(W2, a2, HID)

    in_maps = []
    for c in range(N_CORES):
        base = c * nloc
        src_pad = np.zeros(tot_slots, dtype=np.int64)
        dstl_pad = np.full(tot_slots, 30000, dtype=np.int64)
        off = 0
        for t in range(T):
            n0 = base + t * P
            n1 = min(base + (t + 1) * P, (c + 1) * nloc)
            lo = np.searchsorted(dst_s, n0, side="left")
            hi = np.searchsorted(dst_s, n1, side="left")
            k = hi - lo
            sub = np.argsort(src_s[lo:hi], kind="stable")  # gather locality
            src_pad[off:off + k] = src_s[lo:hi][sub]
            dstl_pad[off:off + k] = (dst_s[lo:hi][sub] - n0)
            off += SLOTS[t]
        assert off == tot_slots

        src16 = np.concatenate(
            [_wrap_idx(src_pad[sum(SLOTS[:t]):sum(SLOTS[:t + 1])].astype(np.int16))
             for t in range(T)], axis=1)

        # owned-node si indices: slot t*128+d -> node base+t*128+d (pad -> 0;
        # pads must be VALID indices so gathered rows stay finite)
        own = np.where(np.arange(T * P) < nloc, base + np.arange(T * P), 0)
        siI = _wrap_idx(own.astype(np.int16))

        # one-hot matrices: slot s = chunk g * 128 + partition p
        dstl_pg = dstl_pad.reshape(tot_chunks, P)            # [g, p]
        dgrid = np.arange(P)
        oh = (dstl_pg[:, :, None] == dgrid[None, None, :])   # [g, p, d]
        oh8 = np.ascontiguousarray(
            oh.transpose(1, 0, 2).reshape(P, tot_chunks * P).astype(F8))
        oh28 = np.ascontiguousarray(
            oh.transpose(2, 0, 1).reshape(P, tot_chunks * P).astype(F8))

        in_maps.append({
            "xT": xT,
            "w1c": w1c,
            "w2c": w2c,
            "src16": np.ascontiguousarray(src16),
            "siI": np.ascontiguousarray(siI),
            "oh8": oh8,
            "oh28": oh28,
        })

    meta = {"T": T, "C_t": C_t, "nloc": nloc, "tot_chunks": tot_chunks}
    return in_maps, meta


# ----------------------------------------------------------------------------
# device program
# ----------------------------------------------------------------------------

def build_program(meta, debug=False, stages=5, iters=1, ablate=""):
    import concourse.bacc as bacc
    import concourse.mybir as mybir
    import concourse.tile as tile
    from concourse.library_config import mlp

    dt = mybir.dt
    Alu = mybir.AluOpType
    Act = mybir.ActivationFunctionType
    DR = mybir.MatmulPerfMode.DoubleRow

    T = meta["T"]
    C_t = meta["C_t"]
    nloc = meta["nloc"]
    tot_chunks = meta["tot_chunks"]

    K1 = N_FEAT // P  # 4
    RB = 512

    nc = bacc.Bacc("TRN2", num_devices=N_CORES, num_swdge_queues=2,
                   dynamic_dma_scratch_size=49152)

    xT = nc.dram_tensor("xT", [N_FEAT, N_NODES], dt.bfloat16,
                        kind="ExternalInput")
    w1c = nc.dram_tensor("w1c", [N_FEAT, PCOLS], dt.bfloat16,
                         kind="ExternalInput")
    w2c = nc.dram_tensor("w2c", [HID, PCOLS], dt.bfloat16,
                         kind="ExternalInput")
    src16 = nc.dram_tensor("src16", [P, tot_chunks * 8], dt.int16,
                           kind="ExternalInput")
    siI16 = nc.dram_tensor("siI", [P, T * 8], dt.int16, kind="ExternalInput")
    oh8 = nc.dram_tensor("oh8", [P, tot_chunks * P], dt.float8e4,
                         kind="ExternalInput")
    oh28 = nc.dram_tensor("oh28", [P, tot_chunks * P], dt.float8e4,
                          kind="ExternalInput")

    out_vec = nc.dram_tensor("out_vec", [1, HID], dt.float32,
                             kind="ExternalOutput")
    dbg = {}
    if debug:
        dbg["embloc"] = nc.dram_tensor("dbg_embloc", [nloc, HID], dt.float32,
                                       kind="ExternalOutput")
        dbg["tab"] = nc.dram_tensor("dbg_tab", [P, ROWB], dt.float32,
                                    kind="ExternalOutput")
        dbg["g"] = nc.dram_tensor("dbg_g", [P, HD], dt.float32,
                                  kind="ExternalOutput")
        dbg["sj"] = nc.dram_tensor("dbg_sj", [P, 8], dt.float32,
                                   kind="ExternalOutput")
        dbg["lg"] = nc.dram_tensor("dbg_lg", [P, 144], dt.float32,
                                   kind="ExternalOutput")
        dbg["ex"] = nc.dram_tensor("dbg_ex", [P, 144], dt.float32,
                                   kind="ExternalOutput")
        dbg["S"] = nc.dram_tensor("dbg_S", [P, HID], dt.float32,
                                  kind="ExternalOutput")
        dbg["sig"] = nc.dram_tensor("dbg_sig", [P, 16], dt.float32,
                                    kind="ExternalOutput")

    table1 = nc.dram_tensor("table1", [N_NODES, ROWB], dt.float8e4)
    sit1 = nc.dram_tensor("sit1", [N_NODES, SIROW], dt.float8e4)
    table2 = nc.dram_tensor("table2", [N_NODES, ROWB], dt.float8e4)
    sit2 = nc.dram_tensor("sit2", [N_NODES, SIROW], dt.float8e4)
    HN = 640  # AG half-split: tiles 0-4 -> A (owned nodes 0:640), 5-9 -> B
    emb_locA = nc.dram_tensor("emb_locA", [HN, HID], dt.bfloat16)
    emb_locB = nc.dram_tensor("emb_locB", [HN, HID], dt.bfloat16)
    emb_locTA = nc.dram_tensor("emb_locTA", [P, HN], dt.bfloat16)
    emb_locTB = nc.dram_tensor("emb_locTB", [P, HN], dt.bfloat16)
    emb_allTA = nc.dram_tensor("emb_allTA", [N_CORES * P, HN], dt.bfloat16,
                               addr_space="Shared")
    emb_allTB = nc.dram_tensor("emb_allTB", [N_CORES * P, HN], dt.bfloat16,
                               addr_space="Shared")

    with tile.TileContext(nc) as tc:
        with (
            tc.tile_pool(name="const", bufs=1) as cpool,
            tc.tile_pool(name="xtp", bufs=8) as tpool,
            tc.tile_pool(name="work", bufs=2) as wpool,
            tc.tile_pool(name="chunk", bufs=4) as kpool,
            tc.tile_pool(name="post", bufs=2) as opool,
            tc.tile_pool(name="psum", bufs=2, space="PSUM") as pspool,
            tc.tile_pool(name="psg", bufs=1, space="PSUM") as psg,
        ):
            nc.gpsimd.load_library(mlp)

            # ---- constants (loaded once per NEFF) ----
            ones_col = cpool.tile([P, 1], dt.bfloat16)
            nc.gpsimd.memset(ones_col[:], 1.0)
            nlog16 = cpool.tile([P, 1], dt.float32)
            nc.gpsimd.memset(nlog16[:], -2.772588722239781)

            w1s = cpool.tile([P, K1 * PCOLS], dt.bfloat16)
            nc.sync.dma_start(
                out=w1s[:].rearrange("p (k c) -> p k c", k=K1),
                in_=w1c[:].rearrange("(k p) c -> p k c", p=P))
            w2s = cpool.tile([P, PCOLS], dt.bfloat16)
            nc.sync.dma_start(out=w2s[:], in_=w2c[:])

            srcI = cpool.tile([P, tot_chunks * 8], dt.int16)
            nc.sync.dma_start(out=srcI[:], in_=src16[:])
            siIS = cpool.tile([P, T * 8], dt.int16)
            nc.sync.dma_start(out=siIS[:], in_=siI16[:])
            ohS = cpool.tile([P, tot_chunks * P], dt.float8e4)
            nc.scalar.dma_start(out=ohS[:], in_=oh8[:])
            oh2S = cpool.tile([P, tot_chunks * P], dt.float8e4)
            nc.scalar.dma_start(out=oh2S[:], in_=oh28[:])

            # ---------------- projection ----------------
            # blocks: list of (r0 = abs node base, rn, src_tensor, prow, pcol)
            # src: lhsT slab src_tensor[prow:prow+128, pcol:pcol+rn]; for
            # layer 1 (K=4 chunks) src_tensor is None -> xT k-chunks.
            def project(layer, table, sit, blocks):
                K = K1 if layer == 1 else 1
                wtile = w1s if layer == 1 else w2s
                SBMAX = 5
                for (r0, rn, srcT, prow, pcol) in blocks:
                    xTs = []
                    for k in range(K):
                        t_ = tpool.tile([P, SBMAX * P], dt.bfloat16, tag="xT")
                        if layer == 1:
                            eng = nc.sync if k % 2 == 0 else nc.scalar
                            eng.dma_start(out=t_[:, :rn],
                                          in_=xT[k * P:(k + 1) * P, r0:r0 + rn])
                        else:
                            nc.sync.dma_start(
                                out=t_[:, :rn],
                                in_=srcT[prow:prow + P, pcol:pcol + rn])
                        xTs.append(t_)
                    SB = -(-rn // P)
                    row = wpool.tile([P, SBMAX, ROWB], dt.float8e4, tag="row")
                    sirow = wpool.tile([P, SBMAX, 16], dt.float8e4, tag="sirow")
                    for s in range(SB):
                        if s * P >= rn:
                            break
                        nn = min(P, rn - s * P)
                        ps = pspool.tile([P, HD], dt.float32, tag="ps")
                        sd = pspool.tile([P, 192], dt.float32, tag="sd")
                        for k in range(K):
                            lhsT = xTs[k][:, s * P:s * P + nn]
                            rhs = wtile[:, k * PCOLS:(k + 1) * PCOLS] if K > 1 \
                                else wtile[:]
                            st, sp = (k == 0), (k == K - 1)
                            nc.tensor.matmul(ps[:nn, 0:512], lhsT=lhsT,
                                             rhs=rhs[:, 0:512], start=st, stop=sp)
                            nc.tensor.matmul(ps[:nn, 512:1024], lhsT=lhsT,
                                             rhs=rhs[:, 512:1024], start=st,
                                             stop=sp)
                            nc.tensor.matmul(sd[:nn, 0:16], lhsT=lhsT,
                                             rhs=rhs[:, 1024:1040], start=st,
                                             stop=sp)
                        nc.vector.tensor_copy(row[:nn, s, 0:768], ps[:nn, 0:768])
                        nc.scalar.activation(row[:nn, s, 768:1024],
                                             ps[:nn, 768:1024], Act.Copy)
                        nc.vector.tensor_copy(
                            row[:nn, s, 1024:1040].bitcast(dt.bfloat16),
                            sd[:nn, 0:8])
                        # si stored as fp8 hi + fp8 lo residual (full
                        # precision across two accumulating matmuls)
                        nc.vector.tensor_copy(sirow[:nn, s, 0:8], sd[:nn, 8:16])
                        hi32 = kpool.tile([P, 8], dt.float32, tag="hi32")
                        nc.vector.tensor_copy(hi32[:nn, :], sirow[:nn, s, 0:8])
                        lo32 = kpool.tile([P, 8], dt.float32, tag="lo32")
                        nc.vector.tensor_tensor(out=lo32[:nn, :],
                                                in0=sd[:nn, 8:16],
                                                in1=hi32[:nn, :],
                                                op=Alu.subtract)
                        nc.vector.tensor_copy(sirow[:nn, s, 8:16], lo32[:nn, :])
                    if rn % P == 0:
                        nc.scalar.dma_start(
                            out=table[r0:r0 + rn, :].rearrange(
                                "(a p) c -> p a c", p=P),
                            in_=row[:, 0:SB, :])
                        nc.sync.dma_start(
                            out=sit[r0:r0 + rn, 0:16].rearrange(
                                "(a p) c -> p a c", p=P),
                            in_=sirow[:, 0:SB, 0:16])
                    else:
                        for s in range(SB):
                            if s * P >= rn:
                                break
                            nn = min(P, rn - s * P)
                            n0 = r0 + s * P
                            nc.scalar.dma_start(out=table[n0:n0 + nn, :],
                                                in_=row[:nn, s, :])
                            nc.sync.dma_start(out=sit[n0:n0 + nn, 0:16],
                                              in_=sirow[:nn, s, 0:16])

            # ---------------- edge phase ----------------
            def edges(table, sit, layer):
                gps = psg.tile([1, HID], dt.float32, tag="gsum", name="gps") \
                    if layer == 2 else None
                ebuf = cpool.tile([P, T * HID], dt.bfloat16, name="ebuf") \
                    if layer == 2 else None

                # owned-node si rows, partition-aligned: siG[d, t, :] is the
                # si row of node base + t*128 + d (fp8: cols 0:8 hi, 8:16 lo)
                siG = wpool.tile([P, T, SIROW], dt.float8e4, tag="siG")
                nc.gpsimd.dma_gather(
                    siG[:], sit[:], siIS[:], T * P, T * P, SIROW,
                    single_packet=False, queue_num=0)
                if debug and layer == 1:
                    dsg = opool.tile([P, 16], dt.float32, tag="dsg", name="dsg")
                    nc.vector.tensor_copy(dsg[:], siG[:, 0, 0:16])
                    nc.sync.dma_start(out=dbg["sig"][:], in_=dsg[:])

                for t in range(T):
                    C = C_t[t]
                    ioff = sum(C_t[:t])
                    n0t = t * P
                    nn_t = min(P, nloc - n0t)
                    G = wpool.tile([P, C, ROWB], dt.float8e4, tag="G")
                    if ablate != "nogather":
                        Ch = C // 2
                        nc.gpsimd.dma_gather(
                            G[:, 0:Ch, :], table[:],
                            srcI[:, ioff * 8:(ioff + Ch) * 8],
                            Ch * P, Ch * P, ROWB, single_packet=False,
                            queue_num=0)
                        nc.gpsimd.dma_gather(
                            G[:, Ch:C, :], table[:],
                            srcI[:, (ioff + Ch) * 8:(ioff + C) * 8],
                            (C - Ch) * P, (C - Ch) * P, ROWB,
                            single_packet=False, queue_num=1)

                    sd = pspool.tile([P, 192], dt.float32, tag="sd")
                    # distribute dst-side si onto edge slots: per chunk,
                    # sd[slot, c*8:(c+1)*8] = oh2_c.T @ (si_hi + si_lo)
                    if ablate != "nosi":
                        for c in range(C):
                            nc.tensor.matmul(
                                sd[:, c * 8:(c + 1) * 8],
                                lhsT=oh2S[:, (ioff + c) * P:(ioff + c + 1) * P],
                                rhs=siG[:, t, 0:8], start=True, stop=False)
                            nc.tensor.matmul(
                                sd[:, c * 8:(c + 1) * 8],
                                lhsT=oh2S[:, (ioff + c) * P:(ioff + c + 1) * P],
                                rhs=siG[:, t, 8:16], start=False, stop=True)
                    # logits = si + sj; exp(leaky_relu(.)) — logits are O(5),
                    # no max-subtraction needed
                    LG = wpool.tile([P, C, 8], dt.float32, tag="LG")
                    nc.vector.tensor_tensor(
                        out=LG[:],
                        in0=sd[:, 0:C * 8].rearrange("p (c e) -> p c e", c=C),
                        in1=G[:, :, 1024:1040].bitcast(dt.bfloat16),
                        op=Alu.add)
                    MX = wpool.tile([P, C * 8], dt.float32, tag="MX")
                    nc.scalar.activation(MX[:],
                                         LG[:].rearrange("p c e -> p (c e)"),
                                         Act.Lrelu, alpha=0.01)
                    # exp scaled by 1/16 so msg = EX*G stays under the fp8
                    # e4m3 max (240); alpha normalization cancels the scale
                    EX = wpool.tile([P, C * 8], dt.float32, tag="EX")
                    nc.scalar.activation(EX[:], MX[:], Act.Exp,
                                         bias=nlog16[:])
                    EXf8 = wpool.tile([P, C, 8], dt.float8e4, tag="EXf8")
                    nc.vector.tensor_copy(
                        EXf8[:], EX[:].rearrange("p (c e) -> p c e", c=C))
                    if debug and layer == 1 and t == 0:
                        dg = wpool.tile([P, HD], dt.float32, tag="dg", name="dg")
                        nc.vector.tensor_copy(dg[:], G[:, 0, 0:HD])
                        nc.sync.dma_start(out=dbg["g"][:], in_=dg[:])
                        dsj = opool.tile([P, 8], dt.float32, tag="dsj",
                                         name="dsj")
                        nc.vector.tensor_copy(
                            dsj[:], G[:, 0, 1024:1040].bitcast(dt.bfloat16))
                        nc.sync.dma_start(out=dbg["sj"][:], in_=dsj[:])
                        dlg = wpool.tile([P, 144], dt.float32, tag="dlg",
                                         name="dlg")
                        nc.vector.tensor_copy(
                            dlg[:], LG[:].rearrange("p c e -> p (c e)")[:, 0:144])
                        nc.sync.dma_start(out=dbg["lg"][:], in_=dlg[:])
                        dex = wpool.tile([P, 144], dt.float32, tag="dex",
                                         name="dex")
                        nc.vector.tensor_copy(dex[:], EX[:, 0:144])
                        nc.sync.dma_start(out=dbg["ex"][:], in_=dex[:])

                    ps = pspool.tile([P, HD], dt.float32, tag="ps")
                    for cp in range(C // 2):
                        c0 = 2 * cp
                        msg = kpool.tile([P, 2, HD], dt.float8e4, tag="msg")
                        if ablate != "nomsg":
                            for j in (0, 1):
                                c = c0 + j
                                for h in range(HEADS):
                                    if h in (3, 7):
                                        nc.scalar.activation(
                                            msg[:, j, h * HID:(h + 1) * HID],
                                            G[:, c, h * HID:(h + 1) * HID],
                                            Act.Copy,
                                            scale=EX[:, c * 8 + h:c * 8 + h + 1])
                                    else:
                                        nc.vector.tensor_scalar_mul(
                                            msg[:, j, h * HID:(h + 1) * HID],
                                            G[:, c, h * HID:(h + 1) * HID],
                                            EX[:, c * 8 + h:c * 8 + h + 1])
                        ohp = ohS[:, (ioff + c0) * P:(ioff + c0 + 2) * P] \
                            .rearrange("p (two d) -> p two d", two=2)
                        st, sp = (cp == 0), (cp == C // 2 - 1)
                        if ablate != "nomm":
                            nc.tensor.matmul(ps[:, 0:512], lhsT=ohp,
                                             rhs=msg[:, :, 0:512], start=st,
                                             stop=sp, perf_mode=DR)
                            nc.tensor.matmul(ps[:, 512:1024], lhsT=ohp,
                                             rhs=msg[:, :, 512:1024], start=st,
                                             stop=sp, perf_mode=DR)
                        nc.tensor.matmul(sd[:, 176:184], lhsT=ohp,
                                         rhs=EXf8[:, c0:c0 + 2, :], start=st,
                                         stop=sp, perf_mode=DR)

                    # ---- postprocess tile ----
                    den = opool.tile([P, 8], dt.float32, tag="den")
                    nc.vector.tensor_scalar(out=den[:], in0=sd[:, 176:184],
                                            scalar1=float(HEADS), scalar2=1e-30,
                                            op0=Alu.mult, op1=Alu.max)
                    rec = opool.tile([P, 8], dt.float32, tag="rec")
                    nc.vector.reciprocal(rec[:], den[:])
                    Sa = opool.tile([P, HID], dt.float32, tag="Sa")
                    Sb = opool.tile([P, HID], dt.float32, tag="Sb")
                    nc.vector.tensor_scalar_mul(Sa[:], ps[:, 0:HID], rec[:, 0:1])
                    for h in range(1, HEADS):
                        tmp = opool.tile([P, HID], dt.float32, tag="tmp")
                        nc.vector.tensor_scalar_mul(
                            tmp[:], ps[:, h * HID:(h + 1) * HID], rec[:, h:h + 1])
                        a, b = (Sa, Sb) if h % 2 == 1 else (Sb, Sa)
                        nc.vector.tensor_tensor(out=b[:], in0=a[:], in1=tmp[:],
                                                op=Alu.add)
                    S = Sb if HEADS % 2 == 0 else Sa
                    if debug and layer == 1 and t == 0:
                        dS = wpool.tile([P, HID], dt.float32, tag="dS",
                                        name="dS")
                        nc.vector.tensor_copy(dS[:], S[:])
                        nc.sync.dma_start(out=dbg["S"][:], in_=dS[:])
                    # elu(S) = exp(min(S,0)) - 1 + max(S,0)
                    neg = opool.tile([P, HID], dt.float32, tag="neg")
                    nc.vector.tensor_scalar_min(neg[:], S[:], 0.0)
                    en = opool.tile([P, HID], dt.float32, tag="en")
                    nc.scalar.activation(en[:], neg[:], Act.Exp)
                    pos = opool.tile([P, HID], dt.float32, tag="pos")
                    nc.vector.tensor_scalar_max(pos[:], S[:], 0.0)
                    eadd = opool.tile([P, HID], dt.float32, tag="eadd")
                    nc.vector.tensor_tensor(out=eadd[:], in0=en[:], in1=pos[:],
                                            op=Alu.add)
                    if layer == 1:
                        ebf = opool.tile([P, HID], dt.bfloat16, tag="ebf")
                        nc.vector.tensor_scalar_add(ebf[:], eadd[:], -1.0)
                        edst = emb_locA if t < 5 else emb_locB
                        e0 = n0t if t < 5 else n0t - HN
                        nc.sync.dma_start(out=edst[e0:e0 + nn_t, :],
                                          in_=ebf[:nn_t, :])
                        # transpose + AllGather each half as soon as its five
                        # tiles are done; AG-A overlaps edges1 tiles 5-9,
                        # AG-B overlaps proj2's A-half
                        if stages >= 3 and t in (4, T - 1):
                            eloc, elocT, eallT = (
                                (emb_locA, emb_locTA, emb_allTA) if t == 4
                                else (emb_locB, emb_locTB, emb_allTB))
                            trh = wpool.tile([P, HN], dt.bfloat16, tag="trh")
                            nc.sync.dma_start_transpose(trh[:, 0:RB],
                                                        eloc[0:RB, :])
                            nc.sync.dma_start_transpose(trh[:, RB:HN],
                                                        eloc[RB:HN, :])
                            nc.scalar.dma_start(out=elocT[:], in_=trh[:])
                            nc.gpsimd.collective_compute(
                                "AllGather", Alu.bypass,
                                ins=[elocT[:]], outs=[eallT[:]],
                                replica_groups=[list(range(N_CORES))])
                    else:
                        nc.vector.tensor_scalar_add(
                            ebuf[:, t * HID:(t + 1) * HID], eadd[:], -1.0)
                if layer == 2:
                    for t in range(T):
                        nn_t = min(P, nloc - t * P)
                        nc.tensor.matmul(gps[0:1, :], lhsT=ones_col[:nn_t, :],
                                         rhs=ebuf[:nn_t, t * HID:(t + 1) * HID],
                                         start=(t == 0), stop=(t == T - 1))
                return gps

            # ---------------- main flow ----------------
            def zero_out_vec():
                z = opool.tile([1, HID], dt.float32, tag="gout", name="z")
                nc.gpsimd.memset(z[:], 0.0)
                nc.sync.dma_start(out=out_vec[:], in_=z[:])

            def flow():
                blocks1 = [(b * RB, min(RB, N_NODES - b * RB), None, 0, 0)
                           for b in range(-(-N_NODES // RB))]
                project(1, table1, sit1, blocks1)
                if debug:
                    tf8 = wpool.tile([P, ROWB], dt.float8e4, tag="tf8",
                                     name="tf8")
                    nc.sync.dma_start(out=tf8[:], in_=table1[0:P, :])
                    t32 = wpool.tile([P, ROWB], dt.float32, tag="t32",
                                     name="t32")
                    nc.vector.tensor_copy(t32[:], tf8[:])
                    nc.sync.dma_start(out=dbg["tab"][:], in_=t32[:])
                if stages >= 2:
                    edges(table1, sit1, layer=1)
                    if debug:
                        for b in range(-(-nloc // P)):
                            n0 = b * P
                            nn = min(P, nloc - n0)
                            src = emb_locA if n0 < HN else emb_locB
                            o0 = n0 if n0 < HN else n0 - HN
                            te = wpool.tile([P, HID], dt.float32, tag="dbgt")
                            tb = wpool.tile([P, HID], dt.bfloat16, tag="dbgb")
                            nc.sync.dma_start(out=tb[:nn, :],
                                              in_=src[o0:o0 + nn, :])
                            nc.vector.tensor_copy(te[:nn, :], tb[:nn, :])
                            nc.sync.dma_start(out=dbg["embloc"][n0:n0 + nn, :],
                                              in_=te[:nn, :])
                if stages >= 4:
                    blocks2 = []
                    for half, srcT, hn in ((0, emb_allTA, HN),
                                           (1, emb_allTB, nloc - HN)):
                        for cblk in range(N_CORES):
                            blocks2.append((cblk * nloc + half * HN, hn,
                                            srcT, cblk * P, 0))
                    project(2, table2, sit2, blocks2)
                if stages >= 5:
                    gps = edges(table2, sit2, layer=2)
                    gout = opool.tile([1, HID], dt.float32, tag="gout")
                    nc.vector.tensor_copy(gout[:], gps[:])
                    nc.sync.dma_start(out=out_vec[:], in_=gout[:])
                else:
                    zero_out_vec()

            for _it in range(iters):
                flow()

    nc.compile()
    return nc


# ----------------------------------------------------------------------------
# top-level kernel
# ----------------------------------------------------------------------------

_CACHE = {}


def _run_device(in_maps, meta):
    from concourse.bass_utils import run_bass_kernel_spmd
    key = "prog"
    if key not in _CACHE:
        _CACHE[key] = build_program(meta)
    nc = _CACHE[key]
    res = run_bass_kernel_spmd(nc, in_maps, core_ids=list(range(N_CORES)))
    return res


def host_finish(partials, ln_g, ln_b, Wl1, bl1, Wl2, bl2, Wl3, bl3):
    g = partials.sum(axis=0) / np.float64(N_NODES)
    mu = g.mean()
    var = ((g - mu) ** 2).mean()
    gn = (g - mu) / np.sqrt(var + 1e-5) * ln_g + ln_b
    x = Wl1 @ gn + bl1
    x = np.maximum(x, 0.01 * x)
    x = Wl2 @ x + bl2
    x = np.maximum(x, 0.01 * x)
    x = Wl3 @ x + bl3
    return np.maximum(x, 0.0).astype(np.float32)


def kernel(node_features, edge_src, edge_dst, W1, a1, W2, a2,
           ln_g, ln_b, Wl1, bl1, Wl2, bl2, Wl3, bl3):
    node_features = np.asarray(node_features, dtype=np.float32)
    edge_src = np.asarray(edge_src, dtype=np.int32)
    edge_dst = np.asarray(edge_dst, dtype=np.int32)
    in_maps, meta = host_prep(node_features, edge_src, edge_dst,
                              np.asarray(W1, np.float32), np.asarray(a1, np.float32),
                              np.asarray(W2, np.float32), np.asarray(a2, np.float32))
    res = _run_device(in_maps, meta)
    partials = np.stack([res.results[c]["out_vec"][0] for c in range(N_CORES)])
    return host_finish(partials.astype(np.float64),
                       np.asarray(ln_g, np.float64), np.asarray(ln_b, np.float64),
                       np.asarray(Wl1, np.float64), np.asarray(bl1, np.float64),
                       np.asarray(Wl2, np.float64), np.asarray(bl2, np.float64),
                       np.asarray(Wl3, np.float64), np.asarray(bl3, np.float64))
